# revision 50
# baseline (speedup 1.0000x reference)
"""Trainium2 Bass kernel for nn_Attention_49606872268904 (v2, bf16).

Dense causal GQA attention block (B=1, S=2048, D=4096, 32 q-heads, 8 kv-heads,
head_dim=128, rope, causal mask, output projection), tensor-parallel over heads
across 8 NeuronCores: core c owns q-heads 4c..4c+3 and kv-head c. Each core
computes its partial output projection in bf16; a chunked bf16 ReduceScatter
sums partials and leaves each core 1/8 of the rows, assembled on host.

v2 changes vs v1:
- All matmul inputs / DMA / collective payloads in bf16 (PSUM accumulation
  stays fp32). Halves HBM + network bytes; removes fp32r small-free penalty.
- Host pre-arranges every DRAM tensor p-major so each DMA is one contiguous
  descriptor per partition (128 descriptors per DMA instead of 512+).
- wk/wv weights resident in SBUF (loaded once, not per seq group).
- wq and wo share one resident SBUF buffer (wq used only in phase P, wo
  loaded over it during phase A0).
- Softmax epilogue: reciprocal_approx_fast instead of full-precision
  reciprocal (5x faster).
- yT partial buffers are [128, 16384] p-major so ReduceScatter slices are
  partition ranges; last seq group scattered in quarters to shrink the tail.
"""

import numpy as np
import ml_dtypes

import concourse.bass as bass
import concourse.mybir as mybir
import concourse.tile as tile
from concourse import bacc
from concourse.bass_utils import run_bass_kernel_spmd
from concourse.masks import make_identity, make_lower_triangular

B, S, DIM = 1, 2048, 4096
NH, NKV, HD = 32, 8, 128
N_CORES = 8
HPC = NH // N_CORES          # 4 q heads per core
OPC = HPC * HD               # 512 output dims per core
DCH = DIM // 128             # 32 contraction chunks
SW = 512                     # seq group width
NSG = S // SW                # 4 seq groups
SCALE = float(HD) ** -0.5

DT = mybir.dt.float32
BF = mybir.dt.bfloat16
F8 = mybir.dt.float8e4
FP = mybir.ActivationFunctionType
DR = mybir.MatmulPerfMode.DoubleRow
FP8_SCALE = 32.0  # x and wq/wk prescale; rope tables divided by 32*32

_cached = None
last_results = None  # BassKernelResults of the most recent run (for test harness)


def build_program():
    nc = bacc.Bacc(
        "TRN2",
        target_bir_lowering=False,
        debug=False,
        enable_asserts=False,
        num_devices=N_CORES,
    )

    # p-major host layouts: one contiguous run per partition per DMA
    xH = nc.declare_dram_parameter("xH", [128, NSG, DCH * SW], BF, isOutput=False)
    # fp8 copies of x / wq / wk (prescaled by 32) laid out in DoubleRow pairs:
    # free index = t*(2*F) + i*F + f for contraction-chunk pair (2t, 2t+1)
    xF = nc.declare_dram_parameter("xF", [128, NSG, 16 * 2 * SW], F8, isOutput=False)
    wq8 = nc.declare_dram_parameter("wq8", [128, 16 * 2 * OPC], F8, isOutput=False)
    wk8 = nc.declare_dram_parameter("wk8", [128, 16 * 2 * HD], F8, isOutput=False)
    wvH = nc.declare_dram_parameter("wvH", [128, DCH * HD], BF, isOutput=False)
    woH = nc.declare_dram_parameter("woH", [128, DCH * OPC], BF, isOutput=False)
    cos2 = nc.declare_dram_parameter("cos2", [128, S], DT, isOutput=False)
    sinpm = nc.declare_dram_parameter("sinpm", [128, S], DT, isOutput=False)
    # core c's final output rows: y[512c + m*128 + p, s] = y_shard[m, p, s]
    y_out = nc.declare_dram_parameter("y_shard", [4, 128, S], BF, isOutput=True)

    with tile.TileContext(nc) as tc:
        with (
            tc.tile_pool(name="dram", bufs=1, space="DRAM") as dram,
            tc.tile_pool(name="consts", bufs=1) as consts,
            tc.tile_pool(name="persist", bufs=1) as persist,
        ):
            # AllGather staging: per seq group, each core contributes its 4
            # heads' normalized attention outputs [128, 4*SW]; the gather
            # fires right after phase A(st), long before W(st) consumes it
            oT_st = [dram.tile([128, HPC * SW], BF, name=f"oT{r}") for r in range(NSG)]
            oG_st = [
                dram.tile([N_CORES * 128, HPC * SW], BF, name=f"oG{r}")
                for r in range(NSG)
            ]

            identf = consts.tile([128, 128], DT)
            make_identity(nc, identf)
            ident = consts.tile([128, 128], BF)
            nc.vector.tensor_copy(ident, identf)
            # negpad[:, 0:128] = strictly-lower-triangular -1e9 (kv > q within
            # the diagonal 128-block), 0 elsewhere; added into scores via an
            # identity matmul so exp() zeroes masked entries with no DVE hop
            negf = consts.tile([128, 128], DT)
            make_lower_triangular(nc, negf, val=-1e9, diag=False)
            negpad = consts.tile([128, SW], BF)
            nc.gpsimd.memset(negpad, 0.0)
            nc.vector.tensor_copy(negpad[:, 0:128], negf)
            ones_f = consts.tile([128, 1], DT)
            nc.gpsimd.memset(ones_f, 1.0)
            ones_col = consts.tile([128, 1], BF)
            nc.vector.tensor_copy(ones_col, ones_f)
            ones_rf = consts.tile([1, 128], DT)
            nc.gpsimd.memset(ones_rf, 1.0)
            ones_row = consts.tile([1, 128], BF)
            nc.vector.tensor_copy(ones_row, ones_rf)

            cos2_sb = consts.tile([128, S], DT)
            sinpm_sb = consts.tile([128, S], DT)

            KT_sb = persist.tile([128, S], BF)       # K_rot^T, all kv positions
            V_sb = persist.tile([128, S], BF)        # V blocks [kv, hd] at col 128j
            wq8_sb = persist.tile([128, 16 * 2 * OPC], F8)  # resident fp8 wq
            wk8_sb = persist.tile([128, 16 * 2 * HD], F8)   # resident fp8 wk
            vw_sb = persist.tile([128, DCH * HD], BF)       # resident wv^T
            wbig = persist.tile([128, DCH * OPC], BF)       # wo (prefetched in P)
            q_tiles = {}

            # ---------------- Phase P: QKV projections + RoPE ----------------
            with (
                nc.named_scope("phaseP"),
                tc.tile_pool(name="psP", bufs=1, space="PSUM") as psP,
                tc.tile_pool(name="sbP", bufs=1) as sbP,
                tc.tile_pool(name="qpool", bufs=1) as qpool,
            ):
                # fp8 q/k weights on gpsimd (first pair-slice first so the t=0
                # matmul starts ASAP); wv + rope tables on the scalar queue;
                # wo prefetched on gpsimd behind the wq/wk loads
                nc.gpsimd.dma_start(wq8_sb[:, 0:1024], wq8[:, 0:1024])
                nc.gpsimd.dma_start(wk8_sb, wk8[:])
                nc.gpsimd.dma_start(wq8_sb[:, 1024:4096], wq8[:, 1024:4096])
                nc.gpsimd.dma_start(wq8_sb[:, 4096:16384], wq8[:, 4096:16384])
                nc.scalar.dma_start(vw_sb, wvH[:])
                nc.scalar.dma_start(cos2_sb, cos2[:])
                nc.scalar.dma_start(sinpm_sb, sinpm[:])
                for j in range(4):
                    nc.gpsimd.dma_start(
                        wbig[:, j * 4096 : (j + 1) * 4096],
                        woH[:, j * 4096 : (j + 1) * 4096],
                    )
                wq8_v = wq8_sb.rearrange("p (t i o) -> p t i o", i=2, o=OPC)
                wk8_v = wk8_sb.rearrange("p (t i o) -> p t i o", i=2, o=HD)

                for sg in range(NSG):
                    scol = slice(sg * SW, (sg + 1) * SW)
                    q_ps = [
                        psP.tile([128, SW], DT, tag=f"q{h}", name=f"qps_{sg}_{h}")
                        for h in range(HPC)
                    ]
                    k_ps = psP.tile([128, SW], DT, tag="k", name=f"kps_{sg}")
                    v_ps = psP.tile([128, SW], DT, tag="v", name=f"vps_{sg}")
                    # Q/K (fp8 DoubleRow) interleaved with V (bf16) in halves
                    # to smooth DMA demand
                    xf = sbP.tile([128, 16 * 2 * SW], F8, tag="xf", bufs=1, name=f"xf_{sg}")
                    xf_v = xf.rearrange("p (t i s) -> p t i s", i=2, s=SW)
                    for half in range(2):
                        nc.sync.dma_start(
                            xf[:, half * 8192 : (half + 1) * 8192],
                            xF[:, sg, half * 8192 : (half + 1) * 8192],
                        )
                        xg = sbP.tile(
                            [128, 16 * SW], BF, tag="xg", bufs=2, name=f"xg_{sg}_{half}"
                        )
                        base = half * 8192
                        nc.sync.dma_start(
                            xg[:, 0:4096], xH[:, sg, base : base + 4096]
                        )
                        nc.sync.dma_start(
                            xg[:, 4096:8192], xH[:, sg, base + 4096 : base + 8192]
                        )
                        for t in range(8 * half, 8 * half + 8):
                            st = t == 0
                            sp = t == 15
                            rhs8 = xf_v[:, t]
                            for h in range(HPC):
                                nc.tensor.matmul(
                                    q_ps[h],
                                    wq8_v[:, t, :, h * HD : (h + 1) * HD],
                                    rhs8,
                                    start=st,
                                    stop=sp,
                                    perf_mode=DR,
                                )
                            nc.tensor.matmul(
                                k_ps, wk8_v[:, t], rhs8, start=st, stop=sp, perf_mode=DR
                            )
                        for i in range(16):
                            d = 16 * half + i
                            nc.tensor.matmul(
                                v_ps, vw_sb[:, d * HD : (d + 1) * HD],
                                xg[:, i * SW : (i + 1) * SW],
                                start=(d == 0), stop=(d == DCH - 1),
                            )

                    # RoPE: out[0:64] = r*cos - i*sin ; out[64:128] = r*sin + i*cos
                    for h in range(HPC):
                        qsb = qpool.tile([128, SW], BF, name=f"qsb_{sg}_{h}")
                        q_tiles[(sg, h)] = qsb
                        t1 = sbP.tile([128, SW], DT, tag="rt1", bufs=2, name=f"rt1_{sg}_{h}")
                        t2 = sbP.tile([128, SW], DT, tag="rt2", bufs=2, name=f"rt2_{sg}_{h}")
                        nc.vector.tensor_mul(t1, q_ps[h], cos2_sb[:, scol])
                        nc.vector.tensor_mul(t2[0:64], q_ps[h][64:128], sinpm_sb[0:64, scol])
                        nc.vector.tensor_mul(t2[64:128], q_ps[h][0:64], sinpm_sb[64:128, scol])
                        nc.vector.tensor_add(qsb, t1, t2)
                    t1k = sbP.tile([128, SW], DT, tag="rt1", bufs=2, name=f"rt1k_{sg}")
                    t2k = sbP.tile([128, SW], DT, tag="rt2", bufs=2, name=f"rt2k_{sg}")
                    nc.vector.tensor_mul(t1k, k_ps, cos2_sb[:, scol])
                    nc.vector.tensor_mul(t2k[0:64], k_ps[64:128], sinpm_sb[0:64, scol])
                    nc.vector.tensor_mul(t2k[64:128], k_ps[0:64], sinpm_sb[64:128, scol])
                    nc.vector.tensor_add(KT_sb[:, scol], t1k, t2k)

                    # V: evacuate then transpose [hd, kv] -> [kv, hd] blocks
                    vtmp = sbP.tile([128, SW], BF, tag="vtmp", bufs=2, name=f"vtmp_{sg}")
                    nc.scalar.copy(vtmp, v_ps)
                    for jj in range(4):
                        j = 4 * sg + jj
                        tr_ps = psP.tile([128, 128], BF, tag="tr", bufs=2, name=f"trp_{j}")
                        nc.tensor.transpose(tr_ps, vtmp[:, jj * 128 : (jj + 1) * 128], ident)
                        nc.vector.tensor_copy(V_sb[:, j * 128 : (j + 1) * 128], tr_ps)

            # ------- Phases A+W interleaved: attention, then output proj + RS
            with (
                tc.tile_pool(name="psA", bufs=1, space="PSUM") as psA,
                tc.tile_pool(name="sbA", bufs=1) as sbA,
                tc.tile_pool(name="psW", bufs=1, space="PSUM") as psW,
                tc.tile_pool(name="sbW", bufs=1) as sbW,
            ):

                def phase_a(qt):
                    with nc.named_scope(f"phaseA{qt}"):
                        nb = 4 * qt + 4

                        def mk_scores(qt, h, j):
                            """Scores for kv-block j (+ causal mask folded in
                            via identity matmul on diagonal-region blocks).
                            Returns (psum tile, valid column offset)."""
                            kk = j - 4 * qt
                            off = 128 * kk if kk > 0 else 0
                            t = psA.tile([128, SW], DT, tag="s", bufs=2, name=f"sps_{qt}_{h}_{j}")
                            if kk >= 0:
                                nc.tensor.matmul(
                                    t[:, off:], ident, negpad[:, : SW - off],
                                    start=True, stop=False,
                                )
                                nc.tensor.matmul(
                                    t[:, off:],
                                    KT_sb[:, j * 128 : (j + 1) * 128],
                                    q_tiles[(qt, h)][:, off:],
                                    start=False,
                                    stop=True,
                                )
                            else:
                                nc.tensor.matmul(
                                    t,
                                    KT_sb[:, j * 128 : (j + 1) * 128],
                                    q_tiles[(qt, h)],
                                    start=True,
                                    stop=True,
                                )
                            return t, off

                        for h in range(HPC):
                            attn_ps = psA.tile([128, SW], DT, tag="attn", bufs=2, name=f"aps_{qt}_{h}")
                            den_ps = psA.tile([1, SW], DT, tag="den", bufs=1, name=f"dps_{qt}_{h}")
                            s_cur, off_cur = mk_scores(qt, h, 0)
                            for j in range(nb):
                                # issue next block's scores before consuming
                                # this one so the PE never idles on exp()
                                s_nxt = mk_scores(qt, h, j + 1) if j + 1 < nb else None
                                off = off_cur
                                exp_sb = sbA.tile([128, SW], BF, tag="exp", bufs=3, name=f"exp_{qt}_{h}_{j}")
                                nc.scalar.activation(
                                    exp_sb[:, off:], s_cur[:, off:], FP.Exp, scale=SCALE
                                )
                                nc.tensor.matmul(
                                    attn_ps[:, off:],
                                    V_sb[:, j * 128 : (j + 1) * 128],
                                    exp_sb[:, off:],
                                    start=(j == 0),
                                    stop=(j == nb - 1),
                                )
                                nc.tensor.matmul(
                                    den_ps[:, off:],
                                    ones_col,
                                    exp_sb[:, off:],
                                    start=(j == 0),
                                    stop=(j == nb - 1),
                                )
                                if s_nxt is not None:
                                    s_cur, off_cur = s_nxt
                            den_sb = sbA.tile([1, SW], DT, tag="densb", bufs=2, name=f"den_{qt}_{h}")
                            nc.scalar.copy(den_sb, den_ps)
                            rd_sb = sbA.tile([1, SW], DT, tag="rd", bufs=2, name=f"rd_{qt}_{h}")
                            nc.vector.reciprocal_approx_fast(rd_sb, den_sb)
                            rd_bf = sbA.tile([1, SW], BF, tag="rdbf", bufs=2, name=f"rdbf_{qt}_{h}")
                            nc.vector.tensor_copy(rd_bf, rd_sb)
                            # broadcast 1/den across partitions on the PE, then
                            # evacuate (DVE can't take two PSUM operands);
                            # keeps the gpsimd queue free for RS triggers
                            rd_ps = psA.tile([128, SW], DT, tag="rdps", bufs=1, name=f"rdps_{qt}_{h}")
                            nc.tensor.matmul(rd_ps, ones_row, rd_bf, start=True, stop=True)
                            rd_bc = sbA.tile([128, SW], DT, tag="rdbc", bufs=2, name=f"rdbc_{qt}_{h}")
                            nc.vector.tensor_copy(rd_bc, rd_ps)
                            at_sb = sbA.tile([128, SW], BF, tag="at", bufs=3, name=f"at_{qt}_{h}")
                            nc.vector.tensor_mul(at_sb, attn_ps, rd_bc)
                            # store on gpsimd: it immediately precedes the AG
                            # trigger that needs it, and blocks nothing else
                            nc.gpsimd.dma_start(
                                oT_st[qt][:, h * SW : (h + 1) * SW], at_sb
                            )
                        # all 4 heads staged: gather this seq group's attention
                        # outputs from all cores (overlaps later A/W phases)
                        nc.gpsimd.collective_compute(
                            "AllGather",
                            mybir.AluOpType.bypass,
                            replica_groups=[list(range(N_CORES))],
                            ins=[oT_st[qt][:]],
                            outs=[oG_st[qt][:]],
                        )

                def phase_w(st):
                    # y rows [512c + m*128 + p] for seq cols of st, contracting
                    # over ALL 32 heads of the gathered attention outputs
                    with nc.named_scope(f"phaseW{st}"):
                        # gathered o streamed as two half-chunks (r 0-3, 4-7)
                        # so double-buffering fits in SBUF
                        ofc = []
                        for cchunk in range(2):
                            oc_t = sbW.tile(
                                [128, 4 * HPC * SW], BF, tag="of", bufs=2,
                                name=f"of_{st}_{cchunk}",
                            )
                            ofc.append(oc_t)
                            for rl in range(4):
                                r = 4 * cchunk + rl
                                nc.sync.dma_start(
                                    oc_t[:, rl * HPC * SW : (rl + 1) * HPC * SW],
                                    oG_st[st][r * 128 : (r + 1) * 128],
                                )
                        for m in range(4):
                            yp = psW.tile([128, SW], DT, tag="yp", bufs=2, name=f"yp_{st}_{m}")
                            for r in range(N_CORES):
                                for oc in range(HPC):
                                    g = 4 * r + oc
                                    nc.tensor.matmul(
                                        yp,
                                        wbig[:, (m * 32 + g) * HD : (m * 32 + g + 1) * HD],
                                        ofc[r // 4][:, ((r % 4) * HPC + oc) * SW : ((r % 4) * HPC + oc + 1) * SW],
                                        start=(g == 0),
                                        stop=(g == 31),
                                    )
                            ysb = sbW.tile([128, SW], BF, tag="ysb", bufs=3, name=f"ysb_{st}_{m}")
                            if m % 2 == 0:
                                nc.scalar.copy(ysb, yp)
                            else:
                                nc.vector.tensor_copy(ysb, yp)
                            nc.gpsimd.dma_start(
                                y_out[m][:, st * SW : (st + 1) * SW], ysb
                            )

                # all attention first (each A(st) fires its AllGather at its
                # end), then the W phases consume the gathers in order
                for qt in range(NSG):
                    phase_a(qt)
                for st in range(NSG):
                    phase_w(st)

    nc.compile()
    return nc


def _get_program():
    global _cached
    if _cached is None:
        _cached = build_program()
    return _cached


_ROPE_PERM = np.concatenate([np.arange(0, HD, 2), np.arange(1, HD, 2)])
_BF = ml_dtypes.bfloat16
_F8 = ml_dtypes.float8_e4m3fn


def kernel(**inputs):
    x = np.asarray(inputs["x"], np.float32)
    wq = np.asarray(inputs["wq"], np.float32)
    wk = np.asarray(inputs["wk"], np.float32)
    wv = np.asarray(inputs["wv"], np.float32)
    wo = np.asarray(inputs["wo"], np.float32)
    fc = np.asarray(inputs["freqs_cos"], np.float32)
    fs = np.asarray(inputs["freqs_sin"], np.float32)

    # xH[p, sg, d*SW + s] = x[0, sg*SW + s, d*128 + p]
    xr = x.reshape(NSG, SW, DCH, 128)
    xH = np.ascontiguousarray(xr.transpose(3, 0, 2, 1)).astype(_BF).reshape(128, NSG, DCH * SW)
    # fp8 copy in DoubleRow pair layout: xF[p, sg, t, i, s], chunk d = 2t+i
    xr8 = x.reshape(NSG, SW, 16, 2, 128)
    xF = (
        np.ascontiguousarray(xr8.transpose(4, 0, 2, 3, 1)) * FP8_SCALE
    ).astype(_F8).reshape(128, NSG, 16 * 2 * SW)

    cosT = np.ascontiguousarray(fc.T)                        # [64, S]
    sinT = np.ascontiguousarray(fs.T)
    unscale = 1.0 / (FP8_SCALE * FP8_SCALE)
    cos2 = np.concatenate([cosT, cosT], axis=0) * unscale    # [128, S]
    sinpm = np.concatenate([-sinT, sinT], axis=0) * unscale

    in_maps = []
    for c in range(N_CORES):
        wq_c = wq[c * OPC : (c + 1) * OPC].reshape(HPC, HD, DIM)[:, _ROPE_PERM]
        wqT_c = wq_c.reshape(OPC, DIM)                       # [o, in]
        # wq8[p, t, i, o] = wq[o, (2t+i)*128 + p] * 32
        wq8_h = (
            np.ascontiguousarray(wqT_c.reshape(OPC, 16, 2, 128).transpose(3, 1, 2, 0))
            * FP8_SCALE
        ).astype(_F8).reshape(128, 16 * 2 * OPC)
        wkT_c = wk[c * HD : (c + 1) * HD][_ROPE_PERM]        # [o, in]
        wk8_h = (
            np.ascontiguousarray(wkT_c.reshape(HD, 16, 2, 128).transpose(3, 1, 2, 0))
            * FP8_SCALE
        ).astype(_F8).reshape(128, 16 * 2 * HD)
        wvT_c = wv[c * HD : (c + 1) * HD]
        wvH = np.ascontiguousarray(
            wvT_c.reshape(HD, DCH, 128).transpose(2, 1, 0)
        ).astype(_BF).reshape(128, DCH * HD)
        # core c produces y rows [512c, 512c+512): needs those wo ROWS over
        # all 4096 head-dims. woH[j, (m*32+g)*128 + row] = wo[512c+m*128+row,
        # g*128+j]
        wo_c = wo[c * OPC : (c + 1) * OPC, :]                # [512, 4096]
        woH = np.ascontiguousarray(
            wo_c.reshape(4, 128, NH, HD).transpose(3, 0, 2, 1)
        ).astype(_BF).reshape(128, DCH * OPC)
        in_maps.append(
            dict(
                xH=xH, xF=xF, wq8=wq8_h, wk8=wk8_h, wvH=wvH, woH=woH,
                cos2=cos2, sinpm=sinpm,
            )
        )

    nc = _get_program()
    res = run_bass_kernel_spmd(nc, in_maps, list(range(N_CORES)))
    global last_results
    last_results = res

    yT = np.empty((DIM, S), np.float32)
    for c in range(N_CORES):
        shard = np.asarray(res.results[c]["y_shard"]).astype(np.float32)  # [4,128,S]
        yT[c * OPC : (c + 1) * OPC] = shard.reshape(OPC, S)
    return np.ascontiguousarray(yT.T).reshape(B, S, DIM)
